# revision 43
# baseline (speedup 1.0000x reference)
"""BERTgrid generator kernel for Trainium2 (8 NeuronCores, data-parallel over batch).

Computes, per batch item (one per core):
  1. seg_emb[n, d] = mean of embeddings[s, d] over tokens s with seg_indices[s] == n
     (mask-weighted), via one-hot fp32 matmuls on the PE.
  2. grid[d, h, w]  = seg_emb[owner(h, w), d] where owner = highest-index box
     covering (h, w), 0 if none. Priority resolution is done with a signed
     triangular matmul (cover[n] - sum_{m>n} cover[m]) + ReLU -> exclusive
     one-hot, then the gather is a fp32r hi+lo matmul pair (near-fp32 exact).

Inputs (full, unsharded): embeddings [8,512,768] f32, mask [8,512] i32,
seg_indices [8,512] i64, coor [8,128,4] i32, plus scalar ints.
Returns (seg_emb [8,128,768] f32, grid [8,768,128,128] f32) like the reference.
"""

import numpy as np
import ml_dtypes

B, S, D, N = 8, 512, 768, 128
GH, GW = 128, 128
HW = GH * GW
SC = S // 128          # 4 token chunks of 128
DC = D // 128          # 6 d chunks
NCHUNK = HW // 512     # 32 hw chunks of 512
COAL = 2               # hw chunks per output staging tile (2*512 = 1024 -> 512KB DMA)
PSUM_W = 1024          # gather psum tile width (2 chunks, 2 banks)

_cache = {}


def _build_nc(reps=1, io_external=True, gather="f32r_hilo", gtile_bufs=12,
              cover_eng="dve", relu_split=True, coal=COAL, oh2_bufs=8,
              copy_pat="DA"):
    import concourse.tile as tile
    import concourse.mybir as mybir
    from concourse import bacc

    f32 = mybir.dt.float32
    f32r = mybir.dt.float32r
    bf16 = mybir.dt.bfloat16
    Alu = mybir.AluOpType

    nc = bacc.Bacc("TRN2", target_bir_lowering=False)

    out_kind = {"kind": "ExternalOutput"} if io_external else {}
    emb_d = nc.dram_tensor("emb", [S, D], f32, kind="ExternalInput")
    # smalls packs segc | maskc | coor | iota as columns to make one DMA
    smalls_d = nc.dram_tensor("smalls", [128, 12 + 128], f32, kind="ExternalInput")
    wsel_d = nc.dram_tensor("wsel", [128, 128], bf16, kind="ExternalInput")
    seg_out = nc.dram_tensor("seg_out", [N, D], f32, **out_kind)
    grid_out = nc.dram_tensor("grid", [D, HW], f32, **out_kind)
    dummy_out = None
    if not io_external:
        dummy_out = nc.dram_tensor("tiny_out", [128, 1], f32, kind="ExternalOutput")

    with tile.TileContext(nc) as tc:
        with (
            tc.tile_pool(name="const", bufs=1) as constp,
            tc.tile_pool(name="seg", bufs=1) as segp,
            tc.tile_pool(name="cover", bufs=oh2_bufs) as coverp,
            tc.tile_pool(name="oh2", bufs=oh2_bufs) as ohp,
            tc.tile_pool(name="gtile", bufs=gtile_bufs) as gtp,
            tc.tile_pool(name="pbig", bufs=3, space="PSUM") as pbig,
            tc.tile_pool(name="psel", bufs=2, space="PSUM") as psel,
        ):
          def emit_once():
            # ---- input loads ----
            # Two small DMAs first on the SP HWDGE ring; the big embedding
            # loads go on the ACT HWDGE ring so they don't serialize behind
            # or in front of the small ones.
            wsel_t = constp.tile([128, 128], bf16, tag="wsel")
            nc.sync.dma_start(wsel_t[:], wsel_d[:])
            smalls_t = constp.tile([128, 12 + 128], f32, tag="smalls")
            nc.sync.dma_start(smalls_t[:], smalls_d[:])
            segc_t = smalls_t[:, 0:SC]
            maskc_t = smalls_t[:, 4:4 + SC]
            coor_t = smalls_t[:, 8:12]
            iota_t = smalls_t[:, 12:140]
            # emb tiles carry an extra all-ones column (768) so the segment
            # matmuls also produce the counts in psum column 768.
            DE = D + 8
            emb_t = []
            for c in range(SC):
                t = constp.tile([128, DE], f32, tag=f"emb{c}")
                nc.scalar.dma_start(t[:, 0:D], emb_d[c * 128:(c + 1) * 128, :])
                nc.vector.memset(t[:, D:DE], 1.0)
                emb_t.append(t)

            # PE warm-up: junk matmuls as soon as wsel lands, so the HAM
            # clock gate opens before the segment matmuls arrive.
            warm = psel.tile([128, 512], f32, tag="sel")
            for _ in range(16):
                nc.tensor.matmul(warm[:, 0:128], wsel_t[:], wsel_t[:],
                                 start=True, stop=True)

            # ---- segment mean ----
            # onehot1m[s, n] = (iota[n] == seg[s]) * mask[s]  per 128-token
            # chunk, in fp32r so the segment sums run at 1 cycle/row on PE.
            # The embeddings are split hi+lo in fp32r (sum reconstructs fp32
            # to ~2^-26 relative).
            oh1 = []
            for c in range(SC):
                t = constp.tile([128, 128], f32, tag=f"oh1_{c}")
                nc.vector.tensor_scalar(
                    out=t[:], in0=iota_t,
                    scalar1=segc_t[:, c:c + 1], scalar2=maskc_t[:, c:c + 1],
                    op0=Alu.is_equal, op1=Alu.mult,
                )
                oh1.append(t)

            psum_seg = pbig.tile([128, PSUM_W], f32, tag="g")
            for c in range(SC):
                st, sp = (c == 0), (c == SC - 1)
                nc.tensor.matmul(psum_seg[:, 0:512], oh1[c][:],
                                 emb_t[c][:, 0:512], start=st, stop=sp)
                nc.tensor.matmul(psum_seg[:, 512:D + 1], oh1[c][:],
                                 emb_t[c][:, 512:D + 1], start=st, stop=sp)

            counts_sb = constp.tile([128, 1], f32, tag="counts")
            nc.vector.tensor_scalar(out=counts_sb[:], in0=psum_seg[:, D:D + 1],
                                    scalar1=1.0, scalar2=None, op0=Alu.max)
            recip = constp.tile([128, 1], f32, tag="recip")
            nc.vector.reciprocal(recip[:], counts_sb[:])

            # fp32r hi/lo split of seg_emb (hi + lo reconstructs ~fp32
            # exactly). hi and seg_emb are scaled copies straight from PSUM on
            # ACT; only the lo remainder needs the DVE.
            g_dt = bf16 if gather == "bf16" else f32r
            hi = segp.tile([128, D], g_dt, tag="hi")
            nc.scalar.mul(hi[:], psum_seg[:, 0:D], recip[:])
            seg_emb = segp.tile([128, D], f32, tag="seg_emb")
            nc.scalar.mul(seg_emb[:], psum_seg[:, 0:D], recip[:])
            nc.sync.dma_start(seg_out[:], seg_emb[:])
            lo = None
            if gather == "f32r_hilo":
                # lo = (sums*recip) - hi in one fused DVE op; the fp32 product
                # is bit-identical to the one hi was rounded from.
                lo = segp.tile([128, D], f32r, tag="lo")
                nc.vector.scalar_tensor_tensor(
                    out=lo[:], in0=psum_seg[:, 0:D], scalar=recip[:],
                    in1=hi[:].bitcast(f32), op0=Alu.mult, op1=Alu.subtract)

            # ---- box cover masks (on the otherwise-idle GPSIMD engine) ----
            # wcover[n, w] = (w >= x0[n]) & (w < x1[n]);  hcover[n, h] likewise
            mask_eng = nc.vector if cover_eng == "dve" else nc.gpsimd
            wa = constp.tile([128, 128], f32, tag="wa")
            mask_eng.tensor_scalar(out=wa[:], in0=iota_t, scalar1=coor_t[:, 0:1],
                                    scalar2=0.0, op0=Alu.subtract, op1=Alu.is_ge)
            wb = constp.tile([128, 128], f32, tag="wb")
            mask_eng.tensor_scalar(out=wb[:], in0=iota_t, scalar1=coor_t[:, 2:3],
                                    scalar2=0.0, op0=Alu.subtract, op1=Alu.is_lt)
            wcover = constp.tile([128, 128], bf16, tag="wcover")
            mask_eng.tensor_tensor(out=wcover[:], in0=wa[:], in1=wb[:], op=Alu.mult)

            ha = constp.tile([128, 128], f32, tag="ha")
            mask_eng.tensor_scalar(out=ha[:], in0=iota_t, scalar1=coor_t[:, 1:2],
                                    scalar2=0.0, op0=Alu.subtract, op1=Alu.is_ge)
            hb = constp.tile([128, 128], f32, tag="hb")
            mask_eng.tensor_scalar(out=hb[:], in0=iota_t, scalar1=coor_t[:, 3:4],
                                    scalar2=0.0, op0=Alu.subtract, op1=Alu.is_lt)
            hcover = constp.tile([128, 128], f32, tag="hcover")
            mask_eng.tensor_tensor(out=hcover[:], in0=ha[:], in1=hb[:], op=Alu.mult)

            # cover chunk j (hw columns [512j, 512j+512)) as its own tile so
            # consumers only depend on the 4 ops that wrote it:
            # cover[n, h*128 + w] = hcover[n, h] * wcover[n, w]
            cover_tiles = [None] * NCHUNK

            def make_cover(j):
                cov = coverp.tile([128, 512], bf16, tag="cov")
                if cover_eng == "mix":
                    eng = nc.gpsimd if j % 2 == 0 else nc.vector
                elif cover_eng == "dve":
                    eng = nc.vector
                else:
                    eng = nc.gpsimd
                for hh in range(4):
                    h = 4 * j + hh
                    eng.tensor_scalar(
                        out=cov[:, hh * GW:(hh + 1) * GW], in0=wcover[:],
                        scalar1=hcover[:, h:h + 1], scalar2=None, op0=Alu.mult,
                    )
                cover_tiles[j] = cov

            # ---- exclusive one-hot (priority = highest box index) ----
            # sel[n, j] = cover[n, j] - sum_{m>n} cover[m, j];  oh2 = relu(sel).
            # Emitted lazily inside the first gather pass so the PE FIFO
            # doesn't drain all 32 sel matmuls (gated by 2 PSUM slots + serial
            # relus) before the first gather matmul.
            oh2 = [None] * NCHUNK

            def make_oh2(j):
                if cover_tiles[j] is None:
                    make_cover(j)
                sp = psel.tile([128, 512], f32, tag="sel")
                nc.tensor.matmul(sp[:], wsel_t[:], cover_tiles[j][:],
                                 start=True, stop=True)
                o = ohp.tile([128, 512], g_dt, tag="oh2")
                if relu_split and j % 2 == 1:
                    nc.vector.tensor_scalar(out=o[:], in0=sp[:], scalar1=0.0,
                                            scalar2=None, op0=Alu.max)
                else:
                    nc.scalar.activation(o[:], sp[:],
                                         mybir.ActivationFunctionType.Relu)
                oh2[j] = o

            # ---- gather matmuls + store ----
            # g-outer / dc-inner: each g-block produces 6 gtiles (one per
            # 128-d slice) from the same 4 one-hot chunks, so the one-hot
            # production cost amortizes across 17us of DMA per block.
            copy_flip = 0
            for g in range(NCHUNK // coal):
                for dc in range(DC):
                    hi_sl = hi[:, dc * 128:(dc + 1) * 128]
                    lo_sl = lo[:, dc * 128:(dc + 1) * 128] if lo is not None else None
                    gt = gtp.tile([128, coal * 512], f32, tag="gt")
                    for half in range(coal * 512 // PSUM_W):
                        ps = pbig.tile([128, PSUM_W], f32, tag="g")
                        for k in range(PSUM_W // 512):
                            j = g * coal + half * (PSUM_W // 512) + k
                            if oh2[j] is None:
                                make_oh2(j)
                            rng = ps[:, k * 512:(k + 1) * 512]
                            if gather == "none":
                                continue
                            if lo_sl is not None:
                                nc.tensor.matmul(rng, hi_sl, oh2[j][:],
                                                 start=True, stop=False)
                                nc.tensor.matmul(rng, lo_sl, oh2[j][:],
                                                 start=False, stop=True)
                            else:
                                nc.tensor.matmul(rng, hi_sl, oh2[j][:],
                                                 start=True, stop=True)
                        dst = gt[:, half * PSUM_W:(half + 1) * PSUM_W]
                        if copy_pat[copy_flip % len(copy_pat)] == "D":
                            nc.vector.tensor_copy(out=dst, in_=ps[:])
                        else:
                            nc.scalar.copy(dst, ps[:])
                        copy_flip += 1
                    nc.sync.dma_start(
                        grid_out[dc * 128:(dc + 1) * 128,
                                 g * coal * 512:(g + 1) * coal * 512],
                        gt[:],
                    )

          if reps == 1:
            emit_once()
          else:
            with tc.For_i(0, reps, 1):
                emit_once()
          if dummy_out is not None:
            dt_ = constp.tile([128, 1], f32, tag="dummy")
            nc.vector.memset(dt_[:], 0.0)
            nc.sync.dma_start(dummy_out[:], dt_[:])

    nc.compile()
    return nc


def _get_nc():
    if "nc" not in _cache:
        _cache["nc"] = _build_nc()
    return _cache["nc"]


def make_in_maps(embeddings, mask, seg_indices, coor):
    iota = np.broadcast_to(np.arange(128, dtype=np.float32), (128, 128))
    wsel = (np.eye(128, dtype=np.float32)
            - np.tril(np.ones((128, 128), dtype=np.float32), -1)
            ).astype(ml_dtypes.bfloat16)
    emb = np.ascontiguousarray(embeddings, dtype=np.float32)
    segc = np.asarray(seg_indices).astype(np.float32).reshape(B, SC, 128)
    maskc = np.asarray(mask).astype(np.float32).reshape(B, SC, 128)
    coorf = np.asarray(coor).astype(np.float32)
    in_maps = []
    for b in range(B):
        smalls = np.concatenate(
            [segc[b].T, maskc[b].T, coorf[b], iota], axis=1)
        in_maps.append({
            "emb": emb[b],
            "smalls": np.ascontiguousarray(smalls),
            "wsel": wsel,
        })
    return in_maps


def kernel(embeddings, mask, seg_indices, coor, num_segments=N, grid_h=GH,
           grid_w=GW, **_ignored):
    from concourse.bass_utils import run_bass_kernel_spmd

    assert int(num_segments) == N and int(grid_h) == GH and int(grid_w) == GW
    nc = _get_nc()
    in_maps = make_in_maps(embeddings, mask, seg_indices, coor)
    res = run_bass_kernel_spmd(nc, in_maps, core_ids=list(range(B)))
    seg_emb = np.stack([res.results[b]["seg_out"] for b in range(B)])
    grid = np.stack([res.results[b]["grid"] for b in range(B)])
    return seg_emb.astype(np.float32), grid.reshape(B, D, GH, GW).astype(np.float32)


# revision 44
# speedup vs baseline: 1.0172x; 1.0172x over previous
"""BERTgrid generator kernel for Trainium2 (8 NeuronCores, data-parallel over batch).

Computes, per batch item (one per core):
  1. seg_emb[n, d] = mean of embeddings[s, d] over tokens s with seg_indices[s] == n
     (mask-weighted), via one-hot fp32 matmuls on the PE.
  2. grid[d, h, w]  = seg_emb[owner(h, w), d] where owner = highest-index box
     covering (h, w), 0 if none. Priority resolution is done with a signed
     triangular matmul (cover[n] - sum_{m>n} cover[m]) + ReLU -> exclusive
     one-hot, then the gather is a fp32r hi+lo matmul pair (near-fp32 exact).

Inputs (full, unsharded): embeddings [8,512,768] f32, mask [8,512] i32,
seg_indices [8,512] i64, coor [8,128,4] i32, plus scalar ints.
Returns (seg_emb [8,128,768] f32, grid [8,768,128,128] f32) like the reference.
"""

import numpy as np
import ml_dtypes

B, S, D, N = 8, 512, 768, 128
GH, GW = 128, 128
HW = GH * GW
SC = S // 128          # 4 token chunks of 128
DC = D // 128          # 6 d chunks
NCHUNK = HW // 512     # 32 hw chunks of 512
COAL = 2               # hw chunks per output staging tile (2*512 = 1024 -> 512KB DMA)
PSUM_W = 1024          # gather psum tile width (2 chunks, 2 banks)

_cache = {}


def _build_nc(reps=1, io_external=True, gather="f32r_hilo", gtile_bufs=12,
              cover_eng="dve", relu_split=True, coal=COAL, oh2_bufs=8,
              copy_pat="DA"):
    import concourse.tile as tile
    import concourse.mybir as mybir
    from concourse import bacc

    f32 = mybir.dt.float32
    f32r = mybir.dt.float32r
    bf16 = mybir.dt.bfloat16
    Alu = mybir.AluOpType

    nc = bacc.Bacc("TRN2", target_bir_lowering=False)

    out_kind = {"kind": "ExternalOutput"} if io_external else {}
    emb_d = nc.dram_tensor("emb", [S, D], f32, kind="ExternalInput")
    # smalls packs segc | maskc | coor | iota as columns to make one DMA
    smalls_d = nc.dram_tensor("smalls", [128, 12 + 128], f32, kind="ExternalInput")
    wsel_d = nc.dram_tensor("wsel", [128, 128], bf16, kind="ExternalInput")
    seg_out = nc.dram_tensor("seg_out", [N, D], f32, **out_kind)
    grid_out = nc.dram_tensor("grid", [D, HW], f32, **out_kind)
    dummy_out = None
    if not io_external:
        dummy_out = nc.dram_tensor("tiny_out", [128, 1], f32, kind="ExternalOutput")

    with tile.TileContext(nc) as tc:
        with (
            tc.tile_pool(name="const", bufs=1) as constp,
            tc.tile_pool(name="seg", bufs=1) as segp,
            tc.tile_pool(name="cover", bufs=oh2_bufs) as coverp,
            tc.tile_pool(name="oh2", bufs=oh2_bufs) as ohp,
            tc.tile_pool(name="gtile", bufs=gtile_bufs) as gtp,
            tc.tile_pool(name="pbig", bufs=3, space="PSUM") as pbig,
            tc.tile_pool(name="psel", bufs=2, space="PSUM") as psel,
        ):
          def emit_once():
            # ---- input loads ----
            # Two small DMAs first on the SP HWDGE ring; the big embedding
            # loads go on the ACT HWDGE ring so they don't serialize behind
            # or in front of the small ones.
            wsel_t = constp.tile([128, 128], bf16, tag="wsel")
            nc.sync.dma_start(wsel_t[:], wsel_d[:])
            smalls_t = constp.tile([128, 12 + 128], f32, tag="smalls")
            nc.sync.dma_start(smalls_t[:], smalls_d[:])
            segc_t = smalls_t[:, 0:SC]
            maskc_t = smalls_t[:, 4:4 + SC]
            coor_t = smalls_t[:, 8:12]
            iota_t = smalls_t[:, 12:140]
            # emb tiles carry an extra all-ones column (768) so the segment
            # matmuls also produce the counts in psum column 768.
            DE = D + 8
            emb_t = []
            for c in range(SC):
                t = constp.tile([128, DE], f32, tag=f"emb{c}")
                nc.scalar.dma_start(t[:, 0:D], emb_d[c * 128:(c + 1) * 128, :])
                nc.vector.memset(t[:, D:DE], 1.0)
                emb_t.append(t)

            # PE warm-up: junk matmuls as soon as wsel lands, so the HAM
            # clock gate opens before the segment matmuls arrive.
            warm = psel.tile([128, 512], f32, tag="sel")
            for _ in range(16):
                nc.tensor.matmul(warm[:, 0:128], wsel_t[:], wsel_t[:],
                                 start=True, stop=True)

            # ---- segment mean ----
            # onehot1m[s, n] = (iota[n] == seg[s]) * mask[s]  per 128-token
            # chunk; exact fp32 segment sums via PE matmuls.
            oh1 = []
            for c in range(SC):
                t = constp.tile([128, 128], f32, tag=f"oh1_{c}")
                nc.vector.tensor_scalar(
                    out=t[:], in0=iota_t,
                    scalar1=segc_t[:, c:c + 1], scalar2=maskc_t[:, c:c + 1],
                    op0=Alu.is_equal, op1=Alu.mult,
                )
                oh1.append(t)

            psum_seg = pbig.tile([128, PSUM_W], f32, tag="g")
            for c in range(SC):
                st, sp = (c == 0), (c == SC - 1)
                nc.tensor.matmul(psum_seg[:, 0:512], oh1[c][:],
                                 emb_t[c][:, 0:512], start=st, stop=sp)
                nc.tensor.matmul(psum_seg[:, 512:D + 1], oh1[c][:],
                                 emb_t[c][:, 512:D + 1], start=st, stop=sp)

            counts_sb = constp.tile([128, 1], f32, tag="counts")
            nc.vector.tensor_scalar(out=counts_sb[:], in0=psum_seg[:, D:D + 1],
                                    scalar1=1.0, scalar2=None, op0=Alu.max)
            recip = constp.tile([128, 1], f32, tag="recip")
            nc.vector.reciprocal(recip[:], counts_sb[:])

            # fp32r hi/lo split of seg_emb (hi + lo reconstructs ~fp32
            # exactly). hi and seg_emb are scaled copies straight from PSUM on
            # ACT; only the lo remainder needs the DVE.
            g_dt = bf16 if gather == "bf16" else f32r
            hi = segp.tile([128, D], g_dt, tag="hi")
            nc.scalar.mul(hi[:], psum_seg[:, 0:D], recip[:])
            seg_emb = segp.tile([128, D], f32, tag="seg_emb")
            nc.scalar.mul(seg_emb[:], psum_seg[:, 0:D], recip[:])
            nc.sync.dma_start(seg_out[:], seg_emb[:])
            lo = None
            if gather == "f32r_hilo":
                # lo = (sums*recip) - hi in one fused DVE op; the fp32 product
                # is bit-identical to the one hi was rounded from.
                lo = segp.tile([128, D], f32r, tag="lo")
                nc.vector.scalar_tensor_tensor(
                    out=lo[:], in0=psum_seg[:, 0:D], scalar=recip[:],
                    in1=hi[:].bitcast(f32), op0=Alu.mult, op1=Alu.subtract)

            # ---- box cover masks ----
            # wcover[n, w] = (w >= x0[n]) & (w < x1[n]);  hcover[n, h] likewise.
            # (GPSIMD is available via cover_eng but is far slower on HW than
            # the cost model claims - keep these on the DVE.)
            mask_eng = nc.vector if cover_eng == "dve" else nc.gpsimd
            wa = constp.tile([128, 128], f32, tag="wa")
            mask_eng.tensor_scalar(out=wa[:], in0=iota_t, scalar1=coor_t[:, 0:1],
                                    scalar2=0.0, op0=Alu.subtract, op1=Alu.is_ge)
            wb = constp.tile([128, 128], f32, tag="wb")
            mask_eng.tensor_scalar(out=wb[:], in0=iota_t, scalar1=coor_t[:, 2:3],
                                    scalar2=0.0, op0=Alu.subtract, op1=Alu.is_lt)
            wcover = constp.tile([128, 128], bf16, tag="wcover")
            mask_eng.tensor_tensor(out=wcover[:], in0=wa[:], in1=wb[:], op=Alu.mult)

            ha = constp.tile([128, 128], f32, tag="ha")
            mask_eng.tensor_scalar(out=ha[:], in0=iota_t, scalar1=coor_t[:, 1:2],
                                    scalar2=0.0, op0=Alu.subtract, op1=Alu.is_ge)
            hb = constp.tile([128, 128], f32, tag="hb")
            mask_eng.tensor_scalar(out=hb[:], in0=iota_t, scalar1=coor_t[:, 3:4],
                                    scalar2=0.0, op0=Alu.subtract, op1=Alu.is_lt)
            hcover = constp.tile([128, 128], f32, tag="hcover")
            mask_eng.tensor_tensor(out=hcover[:], in0=ha[:], in1=hb[:], op=Alu.mult)

            # cover chunk j (hw columns [512j, 512j+512)) as its own tile so
            # consumers only depend on the 4 ops that wrote it:
            # cover[n, h*128 + w] = hcover[n, h] * wcover[n, w]
            cover_tiles = [None] * NCHUNK

            def make_cover(j):
                cov = coverp.tile([128, 512], bf16, tag="cov")
                if cover_eng == "mix":
                    eng = nc.gpsimd if j % 2 == 0 else nc.vector
                elif cover_eng == "dve":
                    eng = nc.vector
                else:
                    eng = nc.gpsimd
                for hh in range(4):
                    h = 4 * j + hh
                    eng.tensor_scalar(
                        out=cov[:, hh * GW:(hh + 1) * GW], in0=wcover[:],
                        scalar1=hcover[:, h:h + 1], scalar2=None, op0=Alu.mult,
                    )
                cover_tiles[j] = cov

            # ---- exclusive one-hot (priority = highest box index) ----
            # sel[n, j] = cover[n, j] - sum_{m>n} cover[m, j];  oh2 = relu(sel).
            # Emitted lazily inside the first gather pass so the PE FIFO
            # doesn't drain all 32 sel matmuls (gated by 2 PSUM slots + serial
            # relus) before the first gather matmul.
            oh2 = [None] * NCHUNK

            def make_oh2(j):
                if cover_tiles[j] is None:
                    make_cover(j)
                sp = psel.tile([128, 512], f32, tag="sel")
                nc.tensor.matmul(sp[:], wsel_t[:], cover_tiles[j][:],
                                 start=True, stop=True)
                o = ohp.tile([128, 512], g_dt, tag="oh2")
                if relu_split and j % 2 == 1:
                    nc.vector.tensor_scalar(out=o[:], in0=sp[:], scalar1=0.0,
                                            scalar2=None, op0=Alu.max)
                else:
                    nc.scalar.activation(o[:], sp[:],
                                         mybir.ActivationFunctionType.Relu)
                oh2[j] = o

            # ---- gather matmuls + store ----
            # g-outer / dc-inner: each g-block produces 6 gtiles (one per
            # 128-d slice) from the same 4 one-hot chunks, so the one-hot
            # production cost amortizes across 17us of DMA per block.
            copy_flip = 0
            for g in range(NCHUNK // coal):
                for dc in range(DC):
                    hi_sl = hi[:, dc * 128:(dc + 1) * 128]
                    lo_sl = lo[:, dc * 128:(dc + 1) * 128] if lo is not None else None
                    gt = gtp.tile([128, coal * 512], f32, tag="gt")
                    for half in range(coal * 512 // PSUM_W):
                        ps = pbig.tile([128, PSUM_W], f32, tag="g")
                        for k in range(PSUM_W // 512):
                            j = g * coal + half * (PSUM_W // 512) + k
                            if oh2[j] is None:
                                make_oh2(j)
                            rng = ps[:, k * 512:(k + 1) * 512]
                            if gather == "none":
                                continue
                            if lo_sl is not None:
                                nc.tensor.matmul(rng, hi_sl, oh2[j][:],
                                                 start=True, stop=False)
                                nc.tensor.matmul(rng, lo_sl, oh2[j][:],
                                                 start=False, stop=True)
                            else:
                                nc.tensor.matmul(rng, hi_sl, oh2[j][:],
                                                 start=True, stop=True)
                        dst = gt[:, half * PSUM_W:(half + 1) * PSUM_W]
                        if copy_pat[copy_flip % len(copy_pat)] == "D":
                            nc.vector.tensor_copy(out=dst, in_=ps[:])
                        else:
                            nc.scalar.copy(dst, ps[:])
                        copy_flip += 1
                    nc.sync.dma_start(
                        grid_out[dc * 128:(dc + 1) * 128,
                                 g * coal * 512:(g + 1) * coal * 512],
                        gt[:],
                    )

          if reps == 1:
            emit_once()
          else:
            with tc.For_i(0, reps, 1):
                emit_once()
          if dummy_out is not None:
            dt_ = constp.tile([128, 1], f32, tag="dummy")
            nc.vector.memset(dt_[:], 0.0)
            nc.sync.dma_start(dummy_out[:], dt_[:])

    nc.compile()
    return nc


def _get_nc():
    if "nc" not in _cache:
        _cache["nc"] = _build_nc()
    return _cache["nc"]


def make_in_maps(embeddings, mask, seg_indices, coor):
    iota = np.broadcast_to(np.arange(128, dtype=np.float32), (128, 128))
    wsel = (np.eye(128, dtype=np.float32)
            - np.tril(np.ones((128, 128), dtype=np.float32), -1)
            ).astype(ml_dtypes.bfloat16)
    emb = np.ascontiguousarray(embeddings, dtype=np.float32)
    segc = np.asarray(seg_indices).astype(np.float32).reshape(B, SC, 128)
    maskc = np.asarray(mask).astype(np.float32).reshape(B, SC, 128)
    coorf = np.asarray(coor).astype(np.float32)
    in_maps = []
    for b in range(B):
        smalls = np.concatenate(
            [segc[b].T, maskc[b].T, coorf[b], iota], axis=1)
        in_maps.append({
            "emb": emb[b],
            "smalls": np.ascontiguousarray(smalls),
            "wsel": wsel,
        })
    return in_maps


def kernel(embeddings, mask, seg_indices, coor, num_segments=N, grid_h=GH,
           grid_w=GW, **_ignored):
    from concourse.bass_utils import run_bass_kernel_spmd

    assert int(num_segments) == N and int(grid_h) == GH and int(grid_w) == GW
    nc = _get_nc()
    in_maps = make_in_maps(embeddings, mask, seg_indices, coor)
    res = run_bass_kernel_spmd(nc, in_maps, core_ids=list(range(B)))
    seg_emb = np.stack([res.results[b]["seg_out"] for b in range(B)])
    grid = np.stack([res.results[b]["grid"] for b in range(B)])
    return seg_emb.astype(np.float32), grid.reshape(B, D, GH, GW).astype(np.float32)
